# revision 17
# baseline (speedup 1.0000x reference)
"""Trainium2 Bass kernel for nn_DetectionPostprocess (nms_detection).

Strategy (pure data parallel over batch, 32 samples per core):
  - Only `cls` is read in full. Per-sample top-20 logits are found with a
    two-level hierarchy built on the DVE max/max_index/match_replace ops
    (top-8 per partition window, then top-24 across the 512 leading
    candidates via 3 match-replace rounds on a PSUM-resident tile).
  - `shape`/`offset` are only touched near the ~20 winning anchors per
    sample: 64-f32 aligned rows fetched with gpsimd dma_gather, then the
    exact element picked with a one-hot multiply+reduce on DVE (the
    within-row offset is f%64 for every tensor because both the sample
    stride 3*13824 and channel stride 13824 are multiples of 64).
    Anchors come from a packed 192-f32-row constant table in the NEFF.
  - Per-partition reorders (candidate->rank inversion, output row
    compaction) use gpsimd local_scatter; cross-partition moves use PE
    transposes and small affine DRAM round-trips.
  - Greedy NMS over the 20 candidate boxes runs as 2 fused DVE ops per
    sequential step on [32, 20] tiles (samples on partitions).
"""

import numpy as np
from contextlib import ExitStack

NCORES = 8
SPC = 32                      # samples per core
DHW = 24
A = DHW * DHW * DHW           # 13824 anchors per sample
P = 128
WCOLS = A // P                # 108 elements per partition window
JMAX = 4                      # per-partition ranks entering level 2
CAND = JMAX * P               # 512 level-2 candidates
NROUND = 3
KX = NROUND * 8               # 24 extracted per sample
K = 20                        # NMS candidate cap (rank < 20)
THRESH = 0.15
NMS_THRESH = 0.05
NEG = -3.0e38

_CACHE = {}


def _build_program(dbg=False):
    import concourse.bacc as bacc
    import concourse.mybir as mybir
    import concourse.tile as tile
    from concourse.masks import make_identity

    f32 = mybir.dt.float32
    u32 = mybir.dt.uint32
    u16 = mybir.dt.uint16
    i16 = mybir.dt.int16
    Alu = mybir.AluOpType
    Act = mybir.ActivationFunctionType

    nc = bacc.Bacc("TRN2", target_bir_lowering=False, debug=False)

    cls_t = nc.dram_tensor("cls", [SPC, A], f32, kind="ExternalInput")
    shp_t = nc.dram_tensor("shp", [SPC * 3 * A], f32, kind="ExternalInput")
    off_t = nc.dram_tensor("off", [SPC * 3 * A], f32, kind="ExternalInput")
    out_t = nc.dram_tensor("out", [SPC, 60, 8], f32, kind="ExternalOutput")

    fa = np.arange(A)
    az = np.zeros((3, 16384), np.float32)
    az[0, :A] = fa // (DHW * DHW)
    az[1, :A] = (fa // DHW) % DHW
    az[2, :A] = fa % DHW
    # packed rows of 192: row r holds [z[64r:64r+64] | y[...] | x[...]]
    anch_np = np.zeros((256, 192), np.float32)
    for c in range(3):
        anch_np[:, c * 64:(c + 1) * 64] = az[c].reshape(256, 64)
    anch_t = nc.inline_tensor(anch_np, "anch")

    with tile.TileContext(nc) as tc, ExitStack() as ctx:
        sb = ctx.enter_context(tc.tile_pool(name="sb", bufs=1))
        ps = ctx.enter_context(tc.tile_pool(name="ps", bufs=1, space="PSUM"))
        dr = ctx.enter_context(tc.tile_pool(name="dr", bufs=1, space="DRAM"))

        # ---- constants -------------------------------------------------
        ident = sb.tile([P, P], f32, tag="ident")
        make_identity(nc, ident[:])

        p108 = sb.tile([P, 1], f32, tag="p108")
        nc.gpsimd.iota(p108[:], pattern=[[0, 1]], base=0, channel_multiplier=WCOLS,
                       allow_small_or_imprecise_dtypes=True)

        # warm the ACT sigmoid table while DMAs run
        warm = sb.tile([SPC, 8], f32, tag="warm")
        nc.gpsimd.memset(warm[:], 0.0)
        nc.scalar.activation(warm[:], warm[:], Act.Sigmoid)

        # ---- phase A: load cls as [128, 32*108] ------------------------
        S = sb.tile([P, SPC * WCOLS], f32, tag="S")
        S_v = S[:].rearrange("p (s c) -> p s c", c=WCOLS)
        cls_v = cls_t[:].rearrange("s (p c) -> p s c", p=P)
        for g in range(8):
            nc.sync.dma_start(
                out=S_v[:, g * 4:(g + 1) * 4, :], in_=cls_v[:, g * 4:(g + 1) * 4, :]
            )

        # ---- phase B: level-1 per-partition top-8 ----------------------
        V8 = sb.tile([P, 8 * SPC], f32, tag="V8")     # col = j*32 + s
        I8 = sb.tile([P, SPC * 8], u32, tag="I8")     # col = s*8 + j
        for s in range(SPC):
            win = S[:, s * WCOLS:(s + 1) * WCOLS]
            nc.vector.max(V8[:, s::SPC], win)
            nc.vector.max_index(I8[:, s * 8:(s + 1) * 8], V8[:, s::SPC], win)

        # ---- phase D: transpose leading ranks into one PSUM bank -------
        Cp = ps.tile([SPC, CAND], f32, tag="Cp")      # col = j*128 + p
        for j in range(JMAX):
            nc.tensor.transpose(
                out=Cp[:, j * P:(j + 1) * P],
                in_=V8[:, j * SPC:(j + 1) * SPC],
                identity=ident[:],
            )

        # ---- phase E: level-2 top-24 via 3 match-replace rounds --------
        vals = sb.tile([SPC, KX], f32, tag="vals")
        pos = sb.tile([SPC, KX], u32, tag="pos")
        for r in range(NROUND):
            nc.vector.max(vals[:, r * 8:(r + 1) * 8], Cp[:])
            nc.vector.max_index(pos[:, r * 8:(r + 1) * 8], vals[:, r * 8:(r + 1) * 8], Cp[:])
            nc.vector.match_replace(Cp[:], vals[:, r * 8:(r + 1) * 8], Cp[:], NEG)

        # ---- phase F: f = p*108 + w per candidate, transposed like vals ----
        F2 = sb.tile([P, JMAX * SPC], f32, tag="F2")   # col = j*32 + s
        F2_v = F2[:].rearrange("p (j s) -> p j s", j=JMAX)
        I8_vv = I8[:].rearrange("p (s j) -> p j s", j=8)[:, 0:JMAX, :]
        nc.vector.tensor_scalar(F2_v, I8_vv, p108[:, 0:1], None, Alu.add)
        Cfp = ps.tile([SPC, CAND], f32, tag="Cfp")
        for j in range(JMAX):
            nc.tensor.transpose(
                out=Cfp[:, j * P:(j + 1) * P],
                in_=F2[:, j * SPC:(j + 1) * SPC],
                identity=ident[:],
            )
        Cf16 = sb.tile([SPC, CAND], u16, tag="Cf16")
        nc.vector.tensor_copy(Cf16[:], Cfp[:])

        # rank-inversion via per-partition local_scatter, then extract f
        pos16 = sb.tile([SPC, KX], i16, tag="pos16")
        nc.vector.tensor_copy(pos16[:], pos[:])
        riota = sb.tile([SPC, KX], i16, tag="riota")
        nc.gpsimd.iota(riota[:], pattern=[[1, KX]], base=1, channel_multiplier=0)
        R = sb.tile([SPC, CAND], i16, tag="R")
        nc.gpsimd.local_scatter(R[:], riota[:], pos16[:], channels=SPC,
                                num_elems=CAND, num_idxs=KX)
        Rm1 = sb.tile([SPC, CAND], i16, tag="Rm1")
        nc.vector.tensor_scalar(Rm1[:], R[:], 1.0, None, Alu.subtract)
        fidx16 = sb.tile([SPC, KX], u16, tag="fidx16")
        nc.gpsimd.local_scatter(fidx16[:], Cf16[:], Rm1[:], channels=SPC,
                                num_elems=KX, num_idxs=CAND)
        fidxf = sb.tile([SPC, KX], f32, tag="fidxf")
        nc.vector.tensor_copy(fidxf[:], fidx16[:])

        # ---- phase G: stable-order fix for duplicated values -----------
        m1 = sb.tile([SPC, 12], u32, tag="m1")
        m2 = sb.tile([SPC, 12], u32, tag="m2")
        tmpf = sb.tile([SPC, 12], f32, tag="tmpf")
        for par in (0, 1):
            npair = (KX - par) // 2
            vE = vals[:, par:par + 2 * npair:2]
            vO = vals[:, par + 1:par + 2 * npair:2]
            fE = fidxf[:, par:par + 2 * npair:2]
            fO = fidxf[:, par + 1:par + 2 * npair:2]
            nc.vector.tensor_tensor(m1[:, :npair], vE, vO, Alu.is_equal)
            nc.vector.tensor_tensor(m2[:, :npair], fE, fO, Alu.is_gt)
            nc.vector.tensor_mul(m1[:, :npair], m1[:, :npair], m2[:, :npair])
            nc.vector.tensor_copy(tmpf[:, :npair], fE)
            nc.vector.copy_predicated(fE, m1[:, :npair], fO)
            nc.vector.copy_predicated(fO, m1[:, :npair], tmpf[:, :npair])

        # ---- phase H: winner tables (r<20), DRAM-roundtripped ----------
        # f%64 / f//64 in exact f32
        fu = sb.tile([SPC, K], u32, tag="fu")
        nc.vector.tensor_copy(fu[:], fidxf[:, :K])
        fmu = sb.tile([SPC, K], u32, tag="fmu")
        nc.vector.tensor_scalar(fmu[:], fu[:], 63, None, Alu.bitwise_and)
        fmf = sb.tile([SPC, K], f32, tag="fmf")
        nc.vector.tensor_copy(fmf[:], fmu[:])
        fdv = sb.tile([SPC, K], f32, tag="fdv")
        nc.vector.scalar_tensor_tensor(fdv[:], fmf[:], -1.0, fidxf[:, :K],
                                       Alu.mult, Alu.add)
        nc.vector.tensor_scalar(fdv[:], fdv[:], 1.0 / 64.0, None, Alu.mult)
        # rowidx = s*648 + f//64  (same for shp; +216c added per channel later)
        s648 = sb.tile([SPC, 1], f32, tag="s648")
        nc.gpsimd.iota(s648[:], pattern=[[0, 1]], base=0, channel_multiplier=648,
                       allow_small_or_imprecise_dtypes=True)
        rowf = sb.tile([SPC, K], f32, tag="rowf")
        nc.vector.tensor_scalar(rowf[:], fdv[:], s648[:, 0:1], None, Alu.add)
        # pack [rowidx | fdiv] as i16 and round-trip into wrapped layout
        wt = sb.tile([SPC, 2 * K], i16, tag="wt")
        nc.vector.tensor_copy(wt[:, 0:K], rowf[:])
        nc.vector.tensor_copy(wt[:, K:2 * K], fdv[:])
        WT_d = dr.tile([2 * 640], i16, tag="WT_d")
        for t in range(2):
            nc.sync.dma_start(
                out=WT_d[t * 640:(t + 1) * 640].rearrange("(r s) -> s r", s=SPC),
                in_=wt[:, t * K:(t + 1) * K])
        idxw = sb.tile([P, 2 * 40], i16, tag="idxw")
        for t in range(2):
            wtd_r = WT_d[t * 640:(t + 1) * 640].rearrange("(m q) -> q m", q=16)
            for kblk in range(8):
                nc.sync.dma_start(
                    out=idxw[kblk * 16:(kblk + 1) * 16, t * 40:(t + 1) * 40],
                    in_=wtd_r)
        idxw1 = sb.tile([P, 40], i16, tag="idxw1")
        idxw2 = sb.tile([P, 40], i16, tag="idxw2")
        nc.vector.tensor_scalar(idxw1[:], idxw[:, 0:40], 216.0, None, Alu.add)
        nc.vector.tensor_scalar(idxw2[:], idxw[:, 0:40], 432.0, None, Alu.add)

        # f%64 into winner-major [128, 5] layout via round-trip
        FM_d = dr.tile([640], f32, tag="FM_d")
        for r4 in range(4):
            nc.sync.dma_start(
                out=FM_d[r4 * 160:(r4 + 1) * 160].rearrange("(s rq) -> s rq", rq=5),
                in_=fmf[:, r4::4])
        offw = sb.tile([P, 5], f32, tag="offw")
        nc.sync.dma_start(out=offw[:], in_=FM_d[:].rearrange("(p q) -> p q", q=5))

        # ---- phase I: 7 dma_gathers (6x 64-f32 rows + packed anchors) --
        gath = sb.tile([P, 6 * 320], f32, tag="gath")
        srcs = [
            (off_t[:].rearrange("(r e) -> r e", e=64), idxw[:, 0:40]),
            (off_t[:].rearrange("(r e) -> r e", e=64), idxw1[:]),
            (off_t[:].rearrange("(r e) -> r e", e=64), idxw2[:]),
            (shp_t[:].rearrange("(r e) -> r e", e=64), idxw[:, 0:40]),
            (shp_t[:].rearrange("(r e) -> r e", e=64), idxw1[:]),
            (shp_t[:].rearrange("(r e) -> r e", e=64), idxw2[:]),
        ]
        for a, (src_ap, iap) in enumerate(srcs):
            nc.gpsimd.dma_gather(
                out_ap=gath[:, a * 320:(a + 1) * 320].rearrange(
                    "p (q e) -> p q e", e=64),
                in_ap=src_ap,
                idxs_ap=iap,
                num_idxs=640,
                num_idxs_reg=640,
                elem_size=64,
            )
        gatha = sb.tile([P, 5 * 192], f32, tag="gatha")
        nc.gpsimd.dma_gather(
            out_ap=gatha[:].rearrange("p (q e) -> p q e", e=192),
            in_ap=anch_t[:].rearrange("r e -> (r e)").rearrange("(r e) -> r e", e=192),
            idxs_ap=idxw[:, 40:80],
            num_idxs=640,
            num_idxs_reg=640,
            elem_size=192,
        )

        # one-hot extraction on DVE: value at column f%64 of each row
        io64 = sb.tile([P, 320], f32, tag="io64")
        nc.gpsimd.iota(io64[:], pattern=[[0, 5], [1, 64]], base=0,
                       channel_multiplier=0, allow_small_or_imprecise_dtypes=True)
        oneh = sb.tile([P, 320], f32, tag="oneh")
        nc.vector.tensor_tensor(
            oneh[:].rearrange("p (q e) -> p q e", e=64),
            io64[:].rearrange("p (q e) -> p q e", e=64),
            offw[:].unsqueeze(2).to_broadcast([P, 5, 64]), Alu.is_equal)
        Wv = sb.tile([P, 45], f32, tag="Wv")
        prod = sb.tile([P, 6 * 320], f32, tag="prod")
        proda = sb.tile([P, 5 * 192], f32, tag="proda")
        oneh4 = oneh[:].rearrange("p (q e) -> p q e", e=64).unsqueeze(1).to_broadcast([P, 6, 5, 64])
        nc.vector.tensor_tensor(
            prod[:].rearrange("p (a q e) -> p a q e", a=6, e=64),
            gath[:].rearrange("p (a q e) -> p a q e", a=6, e=64),
            oneh4, Alu.mult)
        nc.vector.tensor_reduce(
            Wv[:].rearrange("p (q a) -> p a q", a=9)[:, 0:6, :],
            prod[:].rearrange("p (a q e) -> p a q e", a=6, e=64),
            axis=mybir.AxisListType.X, op=Alu.add)
        oneha = oneh[:].rearrange("p (q e) -> p q e", e=64).unsqueeze(2).to_broadcast([P, 5, 3, 64])
        nc.vector.tensor_tensor(
            proda[:].rearrange("p (q c e) -> p q c e", c=3, e=64),
            gatha[:].rearrange("p (q c e) -> p q c e", c=3, e=64),
            oneha, Alu.mult)
        nc.vector.tensor_reduce(
            Wv[:].rearrange("p (q a) -> p q a", a=9)[:, :, 6:9],
            proda[:].rearrange("p (q c e) -> p q c e", c=3, e=64),
            axis=mybir.AxisListType.X, op=Alu.add)

        # round-trip winner-major -> sample-major [32, 20, 9]
        WV_d = dr.tile([640 * 9], f32, tag="WV_d")
        nc.sync.dma_start(
            out=WV_d[:].rearrange("(q p a) -> p q a", p=P, a=9),
            in_=Wv[:].rearrange("p (q a) -> p q a", a=9))
        B9 = sb.tile([SPC, K * 9], f32, tag="B9")
        nc.sync.dma_start(
            out=B9[:].rearrange("s (r a) -> s r a", a=9),
            in_=WV_d[:].rearrange("(r s a) -> s r a", s=SPC, a=9))
        offg = [B9[:, d::9] for d in range(3)]
        shg = [B9[:, 3 + d::9] for d in range(3)]
        anchd = [B9[:, 6 + d::9] for d in range(3)]

        # ---- phase J: det rows [1, score, cz, cy, cx, sz, sy, sx] ------
        det = sb.tile([SPC, K * 8], f32, tag="det")
        nc.gpsimd.memset(det[:, 0::8], 1.0)
        nc.scalar.activation(det[:, 1::8], vals[:, :K], Act.Sigmoid)

        HL = sb.tile([SPC, 7 * K], f32, tag="HL")     # hz hy hx lz ly lx vol
        tctr = sb.tile([SPC, K], f32, tag="tctr")
        for d in range(3):
            nc.vector.tensor_tensor(tctr[:], anchd[d], offg[d], Alu.add)
            nc.vector.tensor_scalar(det[:, 2 + d::8], tctr[:], 4.0, None, Alu.mult)
            nc.vector.tensor_tensor(HL[:, d * K:(d + 1) * K], det[:, 2 + d::8], shg[d], Alu.add)
            nc.vector.tensor_tensor(HL[:, (3 + d) * K:(4 + d) * K], det[:, 2 + d::8], shg[d], Alu.subtract)
            nc.vector.tensor_scalar(det[:, 5 + d::8], shg[d], 2.0, None, Alu.mult)
        vtmp = sb.tile([SPC, K], f32, tag="vtmp")
        nc.vector.tensor_tensor(vtmp[:], det[:, 5::8], det[:, 6::8], Alu.mult)
        nc.vector.tensor_tensor(HL[:, 6 * K:7 * K], vtmp[:], det[:, 7::8], Alu.mult)

        cand = sb.tile([SPC, K], f32, tag="cand")
        nc.vector.tensor_single_scalar(cand[:], det[:, 1::8], THRESH, Alu.is_gt)

        # ---- phase K: pairwise IoU on [32, 400] ------------------------
        def brA(col):
            return HL[:, col * K:(col + 1) * K].unsqueeze(2).to_broadcast([SPC, K, K])

        def brB(col):
            return HL[:, col * K:(col + 1) * K].unsqueeze(1).to_broadcast([SPC, K, K])

        dz = sb.tile([SPC, K * K], f32, tag="dz")
        dy = sb.tile([SPC, K * K], f32, tag="dy")
        dx = sb.tile([SPC, K * K], f32, tag="dx")
        tt = sb.tile([SPC, K * K], f32, tag="tt")
        for d, dd in enumerate((dz, dy, dx)):
            dv = dd[:].rearrange("s (i j) -> s i j", j=K)
            tv = tt[:].rearrange("s (i j) -> s i j", j=K)
            nc.vector.tensor_tensor(dv, brA(d), brB(d), Alu.min)
            nc.vector.tensor_tensor(tv, brA(3 + d), brB(3 + d), Alu.max)
            nc.vector.tensor_tensor(dd[:], dd[:], tt[:], Alu.subtract)
            nc.vector.tensor_scalar(dd[:], dd[:], 0.0, None, Alu.max)
        inter = dz
        nc.vector.tensor_tensor(inter[:], dz[:], dy[:], Alu.mult)
        nc.vector.tensor_tensor(inter[:], inter[:], dx[:], Alu.mult)
        uni = dy
        uv = uni[:].rearrange("s (i j) -> s i j", j=K)
        nc.vector.tensor_tensor(uv, brA(6), brB(6), Alu.add)
        nc.vector.tensor_tensor(uni[:], uni[:], inter[:], Alu.subtract)
        nc.vector.tensor_scalar(uni[:], uni[:], 1e-8, None, Alu.max)
        rec = dx
        nc.vector.reciprocal(rec[:], uni[:])
        iou = tt
        nc.vector.tensor_tensor(iou[:], inter[:], rec[:], Alu.mult)

        negM = sb.tile([SPC, K * K], f32, tag="negM")
        nc.vector.tensor_scalar(negM[:], iou[:], NMS_THRESH, -1.0, Alu.is_gt, Alu.mult)
        nc.gpsimd.memset(negM[:, 0::K + 1], 0.0)

        # ---- phase L: greedy NMS, 20 sequential steps ------------------
        supp = sb.tile([SPC, K], f32, tag="supp")
        negk = sb.tile([SPC, K], f32, tag="negk")
        nc.gpsimd.memset(supp[:], 0.0)
        for i in range(K):
            nc.vector.scalar_tensor_tensor(
                negk[:, i:i + 1], supp[:, i:i + 1], 1.0, cand[:, i:i + 1],
                Alu.subtract, Alu.mult,
            )
            nc.vector.scalar_tensor_tensor(
                supp[:], negM[:, i * K:(i + 1) * K], negk[:, i:i + 1], supp[:],
                Alu.mult, Alu.max,
            )
        kept = negk
        nc.vector.tensor_scalar(kept[:], negk[:], -1.0, None, Alu.mult)

        # ---- phase M: place rows by rank via local_scatter -------------
        incl = sb.tile([SPC, K], f32, tag="incl")
        nc.vector.tensor_tensor_scan(incl[:], kept[:], kept[:], 0.0, Alu.add, Alu.bypass)
        grow = sb.tile([SPC, K], f32, tag="grow")
        nc.vector.tensor_tensor(grow[:], kept[:], incl[:], Alu.mult)
        nc.vector.tensor_scalar(grow[:], grow[:], 1.0, None, Alu.subtract)
        growbc = sb.tile([SPC, K * 16], f32, tag="growbc")
        nc.vector.tensor_copy(growbc[:].rearrange("s (i x) -> s i x", x=16),
                              grow[:].unsqueeze(2).to_broadcast([SPC, K, 16]))
        xio = sb.tile([SPC, K * 16], f32, tag="xio")
        nc.gpsimd.iota(xio[:], pattern=[[0, K], [1, 16]], base=0,
                       channel_multiplier=0, allow_small_or_imprecise_dtypes=True)
        idxo = sb.tile([SPC, K * 16], i16, tag="idxo")
        nc.vector.scalar_tensor_tensor(idxo[:], growbc[:], 16.0, xio[:],
                                       Alu.mult, Alu.add)
        out480 = sb.tile([SPC, 480], f32, tag="out480")
        nc.gpsimd.local_scatter(out480[:].bitcast(u16), det[:].bitcast(u16),
                                idxo[:], channels=SPC, num_elems=960,
                                num_idxs=320)
        io60 = sb.tile([SPC, 60], f32, tag="io60")
        nc.gpsimd.iota(io60[:], pattern=[[1, 60]], base=0, channel_multiplier=0,
                       allow_small_or_imprecise_dtypes=True)
        mask60 = sb.tile([SPC, 60], f32, tag="mask60")
        nc.vector.tensor_scalar(mask60[:], io60[:], incl[:, K - 1:K], None, Alu.is_lt)
        mask480 = sb.tile([SPC, 480], f32, tag="mask480")
        nc.vector.tensor_copy(mask480[:].rearrange("s (r c) -> s r c", c=8),
                              mask60[:].unsqueeze(2).to_broadcast([SPC, 60, 8]))
        outf = sb.tile([SPC, 480], f32, tag="outf")
        nc.vector.tensor_tensor(outf[:], out480[:], mask480[:], Alu.mult)
        nc.vector.scalar_tensor_tensor(outf[:], mask480[:], 1.0, outf[:],
                                       Alu.subtract, Alu.add)
        nc.sync.dma_start(out=out_t[:].rearrange("s r c -> s (r c)"), in_=outf[:])

    nc.compile()
    return nc


def _get_nc():
    if "nc" not in _CACHE:
        _CACHE["nc"] = _build_program()
    return _CACHE["nc"]


def make_in_maps(cls, shape, offset):
    cls = np.ascontiguousarray(np.asarray(cls, dtype=np.float32)).reshape(256, A)
    shape = np.ascontiguousarray(np.asarray(shape, dtype=np.float32)).reshape(256, 3 * A)
    offset = np.ascontiguousarray(np.asarray(offset, dtype=np.float32)).reshape(256, 3 * A)
    in_maps = []
    for c in range(NCORES):
        sl = slice(c * SPC, (c + 1) * SPC)
        in_maps.append({
            "cls": np.ascontiguousarray(cls[sl]),
            "shp": np.ascontiguousarray(shape[sl].reshape(-1)),
            "off": np.ascontiguousarray(offset[sl].reshape(-1)),
        })
    return in_maps


def kernel(cls, shape, offset, _trace=False):
    from concourse.bass_utils import run_bass_kernel_spmd

    nc = _get_nc()
    in_maps = make_in_maps(cls, shape, offset)
    try:
        res = run_bass_kernel_spmd(
            nc, in_maps, core_ids=list(range(NCORES)), trace=_trace)
    except (ImportError, ModuleNotFoundError):
        # NTFF profiling hook unavailable in this environment
        res = run_bass_kernel_spmd(
            nc, in_maps, core_ids=list(range(NCORES)), trace=False)
    out = np.concatenate([res.results[c]["out"] for c in range(NCORES)], axis=0)
    _CACHE["exec_time_ns"] = res.exec_time_ns
    return out.astype(np.float32)


# revision 25
# speedup vs baseline: 1.1224x; 1.1224x over previous
"""Trainium2 Bass kernel for nn_DetectionPostprocess (nms_detection).

Strategy (pure data parallel over batch, 32 samples per core):
  - Only `cls` is read in full. Per-sample top-20 logits are found with a
    two-level hierarchy built on the DVE max/max_index/match_replace ops
    (top-8 per partition window, then top-24 across the 512 leading
    candidates via 3 match-replace rounds on a PSUM-resident tile).
  - `shape`/`offset` are only touched near the ~20 winning anchors per
    sample: 64-f32 aligned rows fetched with gpsimd dma_gather, then the
    exact element picked with a one-hot multiply+reduce on DVE (the
    within-row offset is f%64 for every tensor because both the sample
    stride 3*13824 and channel stride 13824 are multiples of 64).
    Anchors come from a packed 192-f32-row constant table in the NEFF.
  - Per-partition reorders (candidate->rank inversion, output row
    compaction) use gpsimd local_scatter; cross-partition moves use PE
    transposes and small affine DRAM round-trips.
  - Greedy NMS over the 20 candidate boxes runs as 2 fused DVE ops per
    sequential step on [32, 20] tiles (samples on partitions).
"""

import numpy as np
from contextlib import ExitStack

NCORES = 8
SPC = 32                      # samples per core
DHW = 24
A = DHW * DHW * DHW           # 13824 anchors per sample
P = 128
WCOLS = A // P                # 108 elements per partition window
JMAX = 4                      # per-partition ranks entering level 2
CAND = JMAX * P               # 512 level-2 candidates
NROUND = 3
KX = NROUND * 8               # 24 extracted per sample
K = 20                        # NMS candidate cap (rank < 20)
THRESH = 0.15
NMS_THRESH = 0.05
NEG = -3.0e38

_CACHE = {}


def _build_program(dbg=False):
    import concourse.bacc as bacc
    import concourse.mybir as mybir
    import concourse.tile as tile
    from concourse.masks import make_identity

    f32 = mybir.dt.float32
    u32 = mybir.dt.uint32
    u16 = mybir.dt.uint16
    i16 = mybir.dt.int16
    Alu = mybir.AluOpType
    Act = mybir.ActivationFunctionType

    nc = bacc.Bacc("TRN2", target_bir_lowering=False, debug=False)

    cls_t = nc.dram_tensor("cls", [SPC, A], f32, kind="ExternalInput")
    shp_t = nc.dram_tensor("shp", [SPC * 3 * A], f32, kind="ExternalInput")
    off_t = nc.dram_tensor("off", [SPC * 3 * A], f32, kind="ExternalInput")
    out_t = nc.dram_tensor("out", [SPC, 60, 8], f32, kind="ExternalOutput")

    fa = np.arange(A)
    az = np.zeros((3, 16384), np.float32)
    az[0, :A] = fa // (DHW * DHW)
    az[1, :A] = (fa // DHW) % DHW
    az[2, :A] = fa % DHW
    # packed rows of 192: row r holds [z[64r:64r+64] | y[...] | x[...]]
    anch_np = np.zeros((256, 192), np.float32)
    for c in range(3):
        anch_np[:, c * 64:(c + 1) * 64] = az[c].reshape(256, 64)
    anch_t = nc.inline_tensor(anch_np, "anch")

    with tile.TileContext(nc) as tc, ExitStack() as ctx:
        sb = ctx.enter_context(tc.tile_pool(name="sb", bufs=1))
        ps = ctx.enter_context(tc.tile_pool(name="ps", bufs=1, space="PSUM"))
        dr = ctx.enter_context(tc.tile_pool(name="dr", bufs=1, space="DRAM"))

        # ---- constants -------------------------------------------------
        ident = sb.tile([P, P], f32, tag="ident")
        make_identity(nc, ident[:])

        p108 = sb.tile([P, 1], f32, tag="p108")
        nc.gpsimd.iota(p108[:], pattern=[[0, 1]], base=0, channel_multiplier=WCOLS,
                       allow_small_or_imprecise_dtypes=True)

        neg1c = sb.tile([SPC, 320], f32, tag="neg1c")
        nc.gpsimd.memset(neg1c[:], -1.0)
        nc.scalar.dma_start(
            out=out_t[:, K:60, :].rearrange("s r c -> s (r c)"), in_=neg1c[:])

        # warm the ACT sigmoid table while DMAs run
        warm = sb.tile([SPC, 8], f32, tag="warm")
        nc.gpsimd.memset(warm[:], 0.0)
        nc.scalar.activation(warm[:], warm[:], Act.Sigmoid)

        # ---- phase A: load cls as [128, 32*108] ------------------------
        S = sb.tile([P, SPC * WCOLS], f32, tag="S")
        S_v = S[:].rearrange("p (s c) -> p s c", c=WCOLS)
        cls_v = cls_t[:].rearrange("s (p c) -> p s c", p=P)
        bounds = [0, 2, 6, 12, 19, 26, 32]
        engs = [nc.sync, nc.scalar, nc.sync, nc.scalar, nc.sync, nc.scalar]
        for g in range(6):
            lo, hi = bounds[g], bounds[g + 1]
            engs[g].dma_start(out=S_v[:, lo:hi, :], in_=cls_v[:, lo:hi, :])

        # ---- phase B: level-1 per-partition top-8 ----------------------
        V8 = sb.tile([P, 8 * SPC], f32, tag="V8")     # col = j*32 + s
        I8 = sb.tile([P, SPC * 8], u32, tag="I8")     # col = s*8 + j
        for s in range(SPC):
            win = S[:, s * WCOLS:(s + 1) * WCOLS]
            nc.vector.max(V8[:, s::SPC], win)
            nc.vector.max_index(I8[:, s * 8:(s + 1) * 8], V8[:, s::SPC], win)

        # ---- phase D: transpose leading ranks into one PSUM bank -------
        Cp = ps.tile([SPC, CAND], f32, tag="Cp")      # col = j*128 + p
        for j in range(JMAX):
            nc.tensor.transpose(
                out=Cp[:, j * P:(j + 1) * P],
                in_=V8[:, j * SPC:(j + 1) * SPC],
                identity=ident[:],
            )

        # ---- phase E: level-2 top-24 via 3 match-replace rounds --------
        vals = sb.tile([SPC, KX], f32, tag="vals")
        pos = sb.tile([SPC, KX], u32, tag="pos")
        for r in range(NROUND):
            nc.vector.max(vals[:, r * 8:(r + 1) * 8], Cp[:])
            nc.vector.max_index(pos[:, r * 8:(r + 1) * 8], vals[:, r * 8:(r + 1) * 8], Cp[:])
            nc.vector.match_replace(Cp[:], vals[:, r * 8:(r + 1) * 8], Cp[:], NEG)

        # ---- phase F: f = p*108 + w per candidate, transposed like vals ----
        F2 = sb.tile([P, JMAX * SPC], f32, tag="F2")   # col = j*32 + s
        F2_v = F2[:].rearrange("p (j s) -> p j s", j=JMAX)
        I8_vv = I8[:].rearrange("p (s j) -> p j s", j=8)[:, 0:JMAX, :]
        nc.vector.tensor_scalar(F2_v, I8_vv, p108[:, 0:1], None, Alu.add)
        Cfp = ps.tile([SPC, CAND], f32, tag="Cfp")
        for j in range(JMAX):
            nc.tensor.transpose(
                out=Cfp[:, j * P:(j + 1) * P],
                in_=F2[:, j * SPC:(j + 1) * SPC],
                identity=ident[:],
            )
        Cf16 = sb.tile([SPC, CAND], u16, tag="Cf16")
        nc.vector.tensor_copy(Cf16[:], Cfp[:])

        # rank-inversion via per-partition local_scatter, then extract f
        pos16 = sb.tile([SPC, KX], i16, tag="pos16")
        nc.vector.tensor_copy(pos16[:], pos[:])
        riota = sb.tile([SPC, KX], i16, tag="riota")
        nc.gpsimd.iota(riota[:], pattern=[[1, KX]], base=1, channel_multiplier=0)
        R = sb.tile([SPC, CAND], i16, tag="R")
        nc.gpsimd.local_scatter(R[:], riota[:], pos16[:], channels=SPC,
                                num_elems=CAND, num_idxs=KX)
        Rm1 = sb.tile([SPC, CAND], i16, tag="Rm1")
        nc.vector.tensor_scalar(Rm1[:], R[:], 1.0, None, Alu.subtract)
        fidx16 = sb.tile([SPC, KX], u16, tag="fidx16")
        nc.gpsimd.local_scatter(fidx16[:], Cf16[:], Rm1[:], channels=SPC,
                                num_elems=KX, num_idxs=CAND)
        fidxf = sb.tile([SPC, KX], f32, tag="fidxf")
        nc.vector.tensor_copy(fidxf[:], fidx16[:])

        det = sb.tile([SPC, K * 8], f32, tag="det")
        nc.gpsimd.memset(det[:, 0::8], 1.0)
        nc.scalar.activation(det[:, 1::8], vals[:, :K], Act.Sigmoid)
        cand = sb.tile([SPC, K], f32, tag="cand")
        nc.vector.tensor_single_scalar(cand[:], det[:, 1::8], THRESH, Alu.is_gt)

        # ---- phase G: stable-order fix for duplicated values -----------
        m1 = sb.tile([SPC, 12], u32, tag="m1")
        m2 = sb.tile([SPC, 12], u32, tag="m2")
        tmpf = sb.tile([SPC, 12], f32, tag="tmpf")
        for par in (0, 1):
            npair = (KX - par) // 2
            vE = vals[:, par:par + 2 * npair:2]
            vO = vals[:, par + 1:par + 2 * npair:2]
            fE = fidxf[:, par:par + 2 * npair:2]
            fO = fidxf[:, par + 1:par + 2 * npair:2]
            nc.vector.tensor_tensor(m1[:, :npair], vE, vO, Alu.is_equal)
            nc.vector.tensor_tensor(m2[:, :npair], fE, fO, Alu.is_gt)
            nc.vector.tensor_mul(m1[:, :npair], m1[:, :npair], m2[:, :npair])
            nc.vector.tensor_copy(tmpf[:, :npair], fE)
            nc.vector.copy_predicated(fE, m1[:, :npair], fO)
            nc.vector.copy_predicated(fO, m1[:, :npair], tmpf[:, :npair])

        # ---- phase H: winner tables (r<20), DRAM-roundtripped ----------
        # f%64 / f//64 in exact f32
        fu = sb.tile([SPC, K], u32, tag="fu")
        nc.vector.tensor_copy(fu[:], fidxf[:, :K])
        fmu = sb.tile([SPC, K], u32, tag="fmu")
        nc.vector.tensor_scalar(fmu[:], fu[:], 63, None, Alu.bitwise_and)
        fmf = sb.tile([SPC, K], f32, tag="fmf")
        nc.vector.tensor_copy(fmf[:], fmu[:])
        fdv = sb.tile([SPC, K], f32, tag="fdv")
        nc.vector.scalar_tensor_tensor(fdv[:], fmf[:], -1.0, fidxf[:, :K],
                                       Alu.mult, Alu.add)
        nc.vector.tensor_scalar(fdv[:], fdv[:], 1.0 / 64.0, None, Alu.mult)
        # rowidx = s*648 + f//64  (same for shp; +216c added per channel later)
        s648 = sb.tile([SPC, 1], f32, tag="s648")
        nc.gpsimd.iota(s648[:], pattern=[[0, 1]], base=0, channel_multiplier=648,
                       allow_small_or_imprecise_dtypes=True)
        rowf = sb.tile([SPC, K], f32, tag="rowf")
        nc.vector.tensor_scalar(rowf[:], fdv[:], s648[:, 0:1], None, Alu.add)
        # pack [rowidx, fdiv, fmod] t-interleaved as i16; one round-trip into
        # the wrapped layout (entry i=r*32+s at [i%16, i//16], t innermost)
        wt = sb.tile([SPC, 3 * K], i16, tag="wt")
        nc.vector.tensor_copy(wt[:, 0::3], rowf[:])
        nc.vector.tensor_copy(wt[:, 1::3], fdv[:])
        nc.vector.tensor_copy(wt[:, 2::3], fmf[:])
        WT_d = dr.tile([3 * 640], i16, tag="WT_d")
        nc.sync.dma_start(
            out=WT_d[:].rearrange("(r s t) -> s r t", s=SPC, t=3),
            in_=wt[:].rearrange("s (r t) -> s r t", t=3))
        idxw = sb.tile([P, 3 * 40], i16, tag="idxw")   # col = m*3 + t
        wtd_r = WT_d[:].rearrange("(m q t) -> q m t", q=16, t=3)
        qengs = [nc.sync, nc.scalar]
        for kblk in range(8):
            qengs[kblk % 2].dma_start(
                out=idxw[kblk * 16:(kblk + 1) * 16, :].rearrange(
                    "q (m t) -> q m t", t=3),
                in_=wtd_r)
        idxw3 = sb.tile([P, 120], i16, tag="idxw3")
        nc.vector.tensor_copy(idxw3[:, 0:40], idxw[:, 0::3])
        nc.vector.tensor_scalar(idxw3[:, 40:80], idxw[:, 0::3], 216.0, None, Alu.add)
        nc.vector.tensor_scalar(idxw3[:, 80:120], idxw[:, 0::3], 432.0, None, Alu.add)
        fdivw = sb.tile([P, 40], i16, tag="fdivw")
        nc.vector.tensor_copy(fdivw[:], idxw[:, 1::3])

        # f%64 winner-major [128, 5]: winner (pi, slot) sits in the wrapped
        # table at entry [q=pi%16, m=pi//16 + 8*slot]; pull it with 8 tiny
        # strided DMA reads (one per 16-partition block)
        offw16 = sb.tile([P, 5], i16, tag="offw16")
        wt3 = WT_d[:].rearrange("(m q t) -> q m t", q=16, t=3)
        for b in range(8):
            qengs[(b + 1) % 2].dma_start(out=offw16[b * 16:(b + 1) * 16, :],
                                         in_=wt3[:, b:40:8, 2])
        offw = sb.tile([P, 5], f32, tag="offw")
        nc.vector.tensor_copy(offw[:], offw16[:])

        # ---- phase I: 7 dma_gathers of 64-f32 rows ---------------------
        gath = sb.tile([P, 6 * 320], f32, tag="gath")
        for a, src_ap in enumerate((off_t, shp_t)):
            for c in range(3):
                nc.gpsimd.dma_gather(
                    out_ap=gath[:, (a * 3 + c) * 320:(a * 3 + c + 1) * 320].rearrange(
                        "p (q e) -> p q e", e=64),
                    in_ap=src_ap[:].rearrange("(r e) -> r e", e=64),
                    idxs_ap=idxw3[:, c * 40:(c + 1) * 40],
                    num_idxs=640,
                    num_idxs_reg=640,
                    elem_size=64,
                )
        gatha = sb.tile([P, 5 * 192], f32, tag="gatha")
        nc.gpsimd.dma_gather(
            out_ap=gatha[:].rearrange("p (q e) -> p q e", e=192),
            in_ap=anch_t[:].rearrange("r e -> (r e)").rearrange("(r e) -> r e", e=192),
            idxs_ap=fdivw[:],
            num_idxs=640,
            num_idxs_reg=640,
            elem_size=192,
        )

        # one-hot extraction on DVE: value at column f%64 of each row
        io64 = sb.tile([P, 320], f32, tag="io64")
        nc.gpsimd.iota(io64[:], pattern=[[0, 5], [1, 64]], base=0,
                       channel_multiplier=0, allow_small_or_imprecise_dtypes=True)
        oneh = sb.tile([P, 320], f32, tag="oneh")
        nc.vector.tensor_tensor(
            oneh[:].rearrange("p (q e) -> p q e", e=64),
            io64[:].rearrange("p (q e) -> p q e", e=64),
            offw[:].unsqueeze(2).to_broadcast([P, 5, 64]), Alu.is_equal)
        Wv = sb.tile([P, 45], f32, tag="Wv")
        prod = sb.tile([P, 6 * 320], f32, tag="prod")
        proda = sb.tile([P, 5 * 192], f32, tag="proda")
        oneh4 = oneh[:].rearrange("p (q e) -> p q e", e=64).unsqueeze(1).to_broadcast([P, 6, 5, 64])
        nc.vector.tensor_tensor(
            prod[:].rearrange("p (a q e) -> p a q e", a=6, e=64),
            gath[:].rearrange("p (a q e) -> p a q e", a=6, e=64),
            oneh4, Alu.mult)
        # prod a-dim order is (tensor, channel): a = t*3 + c; slot q inner
        nc.vector.tensor_reduce(
            Wv[:].rearrange("p (q a) -> p a q", a=9)[:, 0:6, :],
            prod[:].rearrange("p (a q e) -> p a q e", a=6, e=64),
            axis=mybir.AxisListType.X, op=Alu.add)
        oneha = oneh[:].rearrange("p (q e) -> p q e", e=64).unsqueeze(2).to_broadcast([P, 5, 3, 64])
        nc.vector.tensor_tensor(
            proda[:].rearrange("p (q c e) -> p q c e", c=3, e=64),
            gatha[:].rearrange("p (q c e) -> p q c e", c=3, e=64),
            oneha, Alu.mult)
        nc.vector.tensor_reduce(
            Wv[:].rearrange("p (q a) -> p q a", a=9)[:, :, 6:9],
            proda[:].rearrange("p (q c e) -> p q c e", c=3, e=64),
            axis=mybir.AxisListType.X, op=Alu.add)

        # round-trip winner-major -> sample-major [32, 20, 9]
        WV_d = dr.tile([640 * 9], f32, tag="WV_d")
        nc.scalar.dma_start(
            out=WV_d[:].rearrange("(q p a) -> p q a", p=P, a=9),
            in_=Wv[:].rearrange("p (q a) -> p q a", a=9))
        B9 = sb.tile([SPC, K * 9], f32, tag="B9")
        nc.sync.dma_start(
            out=B9[:].rearrange("s (r a) -> s r a", a=9),
            in_=WV_d[:].rearrange("(r s a) -> s r a", s=SPC, a=9))
        offg = [B9[:, d::9] for d in range(3)]
        shg = [B9[:, 3 + d::9] for d in range(3)]
        anchd = [B9[:, 6 + d::9] for d in range(3)]

        # ---- phase J: det rows [1, score, cz, cy, cx, sz, sy, sx] ------
        HL = sb.tile([SPC, 7 * K], f32, tag="HL")     # hz hy hx lz ly lx vol
        tctr = sb.tile([SPC, K], f32, tag="tctr")
        for d in range(3):
            nc.vector.tensor_tensor(tctr[:], anchd[d], offg[d], Alu.add)
            nc.vector.tensor_scalar(det[:, 2 + d::8], tctr[:], 4.0, None, Alu.mult)
            nc.vector.tensor_tensor(HL[:, d * K:(d + 1) * K], det[:, 2 + d::8], shg[d], Alu.add)
            nc.vector.tensor_tensor(HL[:, (3 + d) * K:(4 + d) * K], det[:, 2 + d::8], shg[d], Alu.subtract)
            nc.vector.tensor_scalar(det[:, 5 + d::8], shg[d], 2.0, None, Alu.mult)
        vtmp = sb.tile([SPC, K], f32, tag="vtmp")
        nc.vector.tensor_tensor(vtmp[:], det[:, 5::8], det[:, 6::8], Alu.mult)
        nc.vector.tensor_tensor(HL[:, 6 * K:7 * K], vtmp[:], det[:, 7::8], Alu.mult)

        # ---- phase K: pairwise IoU on [32, 400] ------------------------
        def brA(col):
            return HL[:, col * K:(col + 1) * K].unsqueeze(2).to_broadcast([SPC, K, K])

        def brB(col):
            return HL[:, col * K:(col + 1) * K].unsqueeze(1).to_broadcast([SPC, K, K])

        dz = sb.tile([SPC, K * K], f32, tag="dz")
        dy = sb.tile([SPC, K * K], f32, tag="dy")
        dx = sb.tile([SPC, K * K], f32, tag="dx")
        tt = sb.tile([SPC, K * K], f32, tag="tt")
        tt2 = sb.tile([SPC, K * K], f32, tag="tt2")
        tt3 = sb.tile([SPC, K * K], f32, tag="tt3")
        tts = [tt, tt2, tt3]
        for d, dd in enumerate((dz, dy, dx)):
            dv = dd[:].rearrange("s (i j) -> s i j", j=K)
            tv = tts[d][:].rearrange("s (i j) -> s i j", j=K)
            nc.vector.tensor_tensor(dv, brA(d), brB(d), Alu.min)
            nc.vector.tensor_tensor(tv, brA(3 + d), brB(3 + d), Alu.max)
            nc.gpsimd.tensor_tensor(dd[:], dd[:], tts[d][:], Alu.subtract)
            nc.gpsimd.tensor_scalar(dd[:], dd[:], 0.0, None, Alu.max)
        inter = dz
        nc.vector.tensor_tensor(inter[:], dz[:], dy[:], Alu.mult)
        nc.vector.tensor_tensor(inter[:], inter[:], dx[:], Alu.mult)
        uni = dy
        uv = uni[:].rearrange("s (i j) -> s i j", j=K)
        nc.vector.tensor_tensor(uv, brA(6), brB(6), Alu.add)
        nc.vector.tensor_tensor(uni[:], uni[:], inter[:], Alu.subtract)
        nc.vector.tensor_scalar(uni[:], uni[:], 1e-8, None, Alu.max)
        rec = dx
        nc.vector.reciprocal(rec[:], uni[:])
        iou = tts[1]
        nc.vector.tensor_tensor(iou[:], inter[:], rec[:], Alu.mult)

        negM = sb.tile([SPC, K * K], f32, tag="negM")
        nc.vector.tensor_scalar(negM[:], iou[:], NMS_THRESH, -1.0, Alu.is_gt, Alu.mult)
        nc.gpsimd.memset(negM[:, 0::K + 1], 0.0)

        # ---- phase L: greedy NMS, 20 sequential steps ------------------
        supp = sb.tile([SPC, K], f32, tag="supp")
        negk = sb.tile([SPC, K], f32, tag="negk")
        nc.gpsimd.memset(supp[:], 0.0)
        for i in range(K):
            nc.vector.scalar_tensor_tensor(
                negk[:, i:i + 1], supp[:, i:i + 1], 1.0, cand[:, i:i + 1],
                Alu.subtract, Alu.mult,
            )
            nc.vector.scalar_tensor_tensor(
                supp[:], negM[:, i * K:(i + 1) * K], negk[:, i:i + 1], supp[:],
                Alu.mult, Alu.max,
            )
        kept = negk
        nc.vector.tensor_scalar(kept[:], negk[:], -1.0, None, Alu.mult)

        # ---- phase M: place rows by rank via local_scatter -------------
        incl = sb.tile([SPC, K], f32, tag="incl")
        nc.vector.tensor_tensor_scan(incl[:], kept[:], kept[:], 0.0, Alu.add, Alu.bypass)
        grow = sb.tile([SPC, K], f32, tag="grow")
        nc.vector.tensor_tensor(grow[:], kept[:], incl[:], Alu.mult)
        nc.vector.tensor_scalar(grow[:], grow[:], 1.0, None, Alu.subtract)
        growbc = sb.tile([SPC, K * 16], f32, tag="growbc")
        nc.scalar.copy(growbc[:].rearrange("s (i x) -> s i x", x=16),
                       grow[:].unsqueeze(2).to_broadcast([SPC, K, 16]))
        xio = sb.tile([SPC, K * 16], f32, tag="xio")
        nc.gpsimd.iota(xio[:], pattern=[[0, K], [1, 16]], base=0,
                       channel_multiplier=0, allow_small_or_imprecise_dtypes=True)
        idxo = sb.tile([SPC, K * 16], i16, tag="idxo")
        nc.vector.scalar_tensor_tensor(idxo[:], growbc[:], 16.0, xio[:],
                                       Alu.mult, Alu.add)
        out160 = sb.tile([SPC, 160], f32, tag="out160")
        nc.gpsimd.local_scatter(out160[:].bitcast(u16), det[:].bitcast(u16),
                                idxo[:], channels=SPC, num_elems=320,
                                num_idxs=320)
        io20 = sb.tile([SPC, K], f32, tag="io20")
        nc.gpsimd.iota(io20[:], pattern=[[1, K]], base=0, channel_multiplier=0,
                       allow_small_or_imprecise_dtypes=True)
        mask20 = sb.tile([SPC, K], f32, tag="mask20")
        nc.vector.tensor_scalar(mask20[:], io20[:], incl[:, K - 1:K], None, Alu.is_lt)
        mask160 = sb.tile([SPC, 160], f32, tag="mask160")
        nc.scalar.copy(mask160[:].rearrange("s (r c) -> s r c", c=8),
                       mask20[:].unsqueeze(2).to_broadcast([SPC, K, 8]))
        outf = sb.tile([SPC, 160], f32, tag="outf")
        nc.vector.tensor_tensor(outf[:], out160[:], mask160[:], Alu.mult)
        nc.vector.scalar_tensor_tensor(outf[:], mask160[:], 1.0,
                                       outf[:], Alu.subtract, Alu.add)
        nc.sync.dma_start(
            out=out_t[:, 0:K, :].rearrange("s r c -> s (r c)"), in_=outf[:])

    nc.compile()
    return nc


def _get_nc():
    if "nc" not in _CACHE:
        _CACHE["nc"] = _build_program()
    return _CACHE["nc"]


def make_in_maps(cls, shape, offset):
    cls = np.ascontiguousarray(np.asarray(cls, dtype=np.float32)).reshape(256, A)
    shape = np.ascontiguousarray(np.asarray(shape, dtype=np.float32)).reshape(256, 3 * A)
    offset = np.ascontiguousarray(np.asarray(offset, dtype=np.float32)).reshape(256, 3 * A)
    in_maps = []
    for c in range(NCORES):
        sl = slice(c * SPC, (c + 1) * SPC)
        in_maps.append({
            "cls": np.ascontiguousarray(cls[sl]),
            "shp": np.ascontiguousarray(shape[sl].reshape(-1)),
            "off": np.ascontiguousarray(offset[sl].reshape(-1)),
        })
    return in_maps


def kernel(cls, shape, offset, _trace=False):
    from concourse.bass_utils import run_bass_kernel_spmd

    nc = _get_nc()
    in_maps = make_in_maps(cls, shape, offset)
    try:
        res = run_bass_kernel_spmd(
            nc, in_maps, core_ids=list(range(NCORES)), trace=_trace)
    except (ImportError, ModuleNotFoundError):
        # NTFF profiling hook unavailable in this environment
        res = run_bass_kernel_spmd(
            nc, in_maps, core_ids=list(range(NCORES)), trace=False)
    out = np.concatenate([res.results[c]["out"] for c in range(NCORES)], axis=0)
    _CACHE["exec_time_ns"] = res.exec_time_ns
    return out.astype(np.float32)


# revision 29
# speedup vs baseline: 1.1894x; 1.0596x over previous
"""Trainium2 Bass kernel for nn_DetectionPostprocess (nms_detection).

Strategy (pure data parallel over batch, 32 samples per core):
  - Only `cls` is read in full. Per-sample top-20 logits are found with a
    two-level hierarchy built on the DVE max/max_index/match_replace ops
    (top-8 per partition window, then top-24 across the 512 leading
    candidates via 3 match-replace rounds on a PSUM-resident tile).
  - `shape`/`offset` are only touched near the ~20 winning anchors per
    sample: 64-f32 aligned rows fetched with gpsimd dma_gather, then the
    exact element picked with a one-hot multiply+reduce on DVE (the
    within-row offset is f%64 for every tensor because both the sample
    stride 3*13824 and channel stride 13824 are multiples of 64).
    Anchors come from a packed 192-f32-row constant table in the NEFF.
  - Per-partition reorders (candidate->rank inversion, output row
    compaction) use gpsimd local_scatter; cross-partition moves use PE
    transposes and small affine DRAM round-trips.
  - Greedy NMS over the 20 candidate boxes runs as 2 fused DVE ops per
    sequential step on [32, 20] tiles (samples on partitions).
"""

import numpy as np
from contextlib import ExitStack

NCORES = 8
SPC = 32                      # samples per core
DHW = 24
A = DHW * DHW * DHW           # 13824 anchors per sample
P = 128
WCOLS = A // P                # 108 elements per partition window
JMAX = 4                      # per-partition ranks entering level 2
CAND = JMAX * P               # 512 level-2 candidates
NROUND = 3
KX = NROUND * 8               # 24 extracted per sample
K = 20                        # NMS candidate cap (rank < 20)
THRESH = 0.15
NMS_THRESH = 0.05
NEG = -3.0e38

_CACHE = {}


def _build_program(dbg=False):
    import concourse.bacc as bacc
    import concourse.mybir as mybir
    import concourse.tile as tile
    from concourse.masks import make_identity

    f32 = mybir.dt.float32
    u32 = mybir.dt.uint32
    u16 = mybir.dt.uint16
    i16 = mybir.dt.int16
    Alu = mybir.AluOpType
    Act = mybir.ActivationFunctionType

    nc = bacc.Bacc("TRN2", target_bir_lowering=False, debug=False)

    cls_t = nc.dram_tensor("cls", [SPC, A], f32, kind="ExternalInput")
    shp_t = nc.dram_tensor("shp", [SPC * 3 * A], f32, kind="ExternalInput")
    off_t = nc.dram_tensor("off", [SPC * 3 * A], f32, kind="ExternalInput")
    out_t = nc.dram_tensor("out", [SPC, 60, 8], f32, kind="ExternalOutput")

    fa = np.arange(A)
    az = np.zeros((3, 16384), np.float32)
    az[0, :A] = fa // (DHW * DHW)
    az[1, :A] = (fa // DHW) % DHW
    az[2, :A] = fa % DHW
    # packed rows of 192: row r holds [z[64r:64r+64] | y[...] | x[...]]
    anch_np = np.zeros((256, 192), np.float32)
    for c in range(3):
        anch_np[:, c * 64:(c + 1) * 64] = az[c].reshape(256, 64)
    anch_t = nc.inline_tensor(anch_np, "anch")

    with tile.TileContext(nc) as tc, ExitStack() as ctx:
        sb = ctx.enter_context(tc.tile_pool(name="sb", bufs=1))
        ps = ctx.enter_context(tc.tile_pool(name="ps", bufs=1, space="PSUM"))
        dr = ctx.enter_context(tc.tile_pool(name="dr", bufs=1, space="DRAM"))

        # ---- constants -------------------------------------------------
        ident = sb.tile([P, P], f32, tag="ident")
        make_identity(nc, ident[:])

        p108 = sb.tile([P, 1], f32, tag="p108")
        nc.gpsimd.iota(p108[:], pattern=[[0, 1]], base=0, channel_multiplier=WCOLS,
                       allow_small_or_imprecise_dtypes=True)

        neg1c = sb.tile([SPC, 320], f32, tag="neg1c")
        nc.gpsimd.memset(neg1c[:], -1.0)
        nc.scalar.dma_start(
            out=out_t[:, K:60, :].rearrange("s r c -> s (r c)"), in_=neg1c[:])

        # warm the ACT sigmoid table while DMAs run
        warm = sb.tile([SPC, 8], f32, tag="warm")
        nc.gpsimd.memset(warm[:], 0.0)
        nc.scalar.activation(warm[:], warm[:], Act.Sigmoid)

        # ---- phase A: load cls as [128, 32*108] ------------------------
        S = sb.tile([P, SPC * WCOLS], f32, tag="S")
        S_v = S[:].rearrange("p (s c) -> p s c", c=WCOLS)
        cls_v = cls_t[:].rearrange("s (p c) -> p s c", p=P)
        bounds = [0, 2, 6, 12, 19, 26, 32]
        engs = [nc.sync, nc.scalar, nc.sync, nc.scalar, nc.sync, nc.scalar]
        for g in range(6):
            lo, hi = bounds[g], bounds[g + 1]
            engs[g].dma_start(out=S_v[:, lo:hi, :], in_=cls_v[:, lo:hi, :])

        # ---- phase B: level-1 per-partition top-8 ----------------------
        V8 = sb.tile([P, 8 * SPC], f32, tag="V8")     # col = j*32 + s
        I8 = sb.tile([P, SPC * 8], u32, tag="I8")     # col = s*8 + j
        for s in range(SPC):
            win = S[:, s * WCOLS:(s + 1) * WCOLS]
            nc.vector.max(V8[:, s::SPC], win)
            nc.vector.max_index(I8[:, s * 8:(s + 1) * 8], V8[:, s::SPC], win)

        # ---- phase D: transpose leading ranks into one PSUM bank -------
        Cp = ps.tile([SPC, CAND], f32, tag="Cp")      # col = j*128 + p
        for j in range(JMAX):
            nc.tensor.transpose(
                out=Cp[:, j * P:(j + 1) * P],
                in_=V8[:, j * SPC:(j + 1) * SPC],
                identity=ident[:],
            )

        # ---- phase E: level-2 top-24 via 3 match-replace rounds --------
        vals = sb.tile([SPC, KX], f32, tag="vals")
        pos = sb.tile([SPC, KX], u32, tag="pos")
        for r in range(NROUND):
            nc.vector.max(vals[:, r * 8:(r + 1) * 8], Cp[:])
            nc.vector.max_index(pos[:, r * 8:(r + 1) * 8], vals[:, r * 8:(r + 1) * 8], Cp[:])
            nc.vector.match_replace(Cp[:], vals[:, r * 8:(r + 1) * 8], Cp[:], NEG)

        # ---- phase F: f = p*108 + w per candidate, transposed like vals ----
        F2 = sb.tile([P, JMAX * SPC], f32, tag="F2")   # col = j*32 + s
        F2_v = F2[:].rearrange("p (j s) -> p j s", j=JMAX)
        I8_vv = I8[:].rearrange("p (s j) -> p j s", j=8)[:, 0:JMAX, :]
        nc.vector.tensor_scalar(F2_v, I8_vv, p108[:, 0:1], None, Alu.add)
        Cfp = ps.tile([SPC, CAND], f32, tag="Cfp")
        for j in range(JMAX):
            nc.tensor.transpose(
                out=Cfp[:, j * P:(j + 1) * P],
                in_=F2[:, j * SPC:(j + 1) * SPC],
                identity=ident[:],
            )
        Cf16 = sb.tile([SPC, CAND], u16, tag="Cf16")
        nc.vector.tensor_copy(Cf16[:], Cfp[:])

        # rank-inversion via per-partition local_scatter, then extract f
        pos16 = sb.tile([SPC, KX], i16, tag="pos16")
        nc.vector.tensor_copy(pos16[:], pos[:])
        riota = sb.tile([SPC, KX], i16, tag="riota")
        nc.gpsimd.iota(riota[:], pattern=[[1, KX]], base=1, channel_multiplier=0)
        R = sb.tile([SPC, CAND], i16, tag="R")
        nc.gpsimd.local_scatter(R[:], riota[:], pos16[:], channels=SPC,
                                num_elems=CAND, num_idxs=KX)
        Rm1 = sb.tile([SPC, CAND], i16, tag="Rm1")
        nc.vector.tensor_scalar(Rm1[:], R[:], 1.0, None, Alu.subtract)
        fidx16 = sb.tile([SPC, KX], u16, tag="fidx16")
        nc.gpsimd.local_scatter(fidx16[:], Cf16[:], Rm1[:], channels=SPC,
                                num_elems=KX, num_idxs=CAND)
        fidxf = sb.tile([SPC, KX], f32, tag="fidxf")
        nc.vector.tensor_copy(fidxf[:], fidx16[:])

        det = sb.tile([SPC, K * 8], f32, tag="det")
        nc.gpsimd.memset(det[:, 0::8], 1.0)
        nc.scalar.activation(det[:, 1::8], vals[:, :K], Act.Sigmoid)
        cand = sb.tile([SPC, K], f32, tag="cand")
        nc.vector.tensor_single_scalar(cand[:], det[:, 1::8], THRESH, Alu.is_gt)

        # ---- phase G: stable-order fix for duplicated values -----------
        m1 = sb.tile([SPC, 12], u32, tag="m1")
        m2 = sb.tile([SPC, 12], u32, tag="m2")
        tmpf = sb.tile([SPC, 12], f32, tag="tmpf")
        for par in (0, 1):
            npair = (KX - par) // 2
            vE = vals[:, par:par + 2 * npair:2]
            vO = vals[:, par + 1:par + 2 * npair:2]
            fE = fidxf[:, par:par + 2 * npair:2]
            fO = fidxf[:, par + 1:par + 2 * npair:2]
            nc.vector.tensor_tensor(m1[:, :npair], vE, vO, Alu.is_equal)
            nc.vector.tensor_tensor(m2[:, :npair], fE, fO, Alu.is_gt)
            nc.vector.tensor_mul(m1[:, :npair], m1[:, :npair], m2[:, :npair])
            nc.vector.tensor_copy(tmpf[:, :npair], fE)
            nc.vector.copy_predicated(fE, m1[:, :npair], fO)
            nc.vector.copy_predicated(fO, m1[:, :npair], tmpf[:, :npair])

        # ---- phase H: winner tables (r<20), DRAM-roundtripped ----------
        # f%64 / f//64 in exact f32
        fu = sb.tile([SPC, K], u32, tag="fu")
        nc.vector.tensor_copy(fu[:], fidxf[:, :K])
        fmu = sb.tile([SPC, K], u32, tag="fmu")
        nc.vector.tensor_scalar(fmu[:], fu[:], 63, None, Alu.bitwise_and)
        fmf = sb.tile([SPC, K], f32, tag="fmf")
        nc.vector.tensor_copy(fmf[:], fmu[:])
        fdv = sb.tile([SPC, K], f32, tag="fdv")
        nc.vector.scalar_tensor_tensor(fdv[:], fmf[:], -1.0, fidxf[:, :K],
                                       Alu.mult, Alu.add)
        nc.vector.tensor_scalar(fdv[:], fdv[:], 1.0 / 64.0, None, Alu.mult)
        # rowidx = s*648 + f//64  (same for shp; +216c added per channel later)
        s648 = sb.tile([SPC, 1], f32, tag="s648")
        nc.gpsimd.iota(s648[:], pattern=[[0, 1]], base=0, channel_multiplier=648,
                       allow_small_or_imprecise_dtypes=True)
        rowf = sb.tile([SPC, K], f32, tag="rowf")
        nc.vector.tensor_scalar(rowf[:], fdv[:], s648[:, 0:1], None, Alu.add)
        # pack [rowidx, fdiv] t-interleaved as i16; one round-trip into the
        # wrapped layout (entry i=r*32+s at [i%16, i//16], t innermost)
        wt = sb.tile([SPC, 2 * K], i16, tag="wt")
        nc.vector.tensor_copy(wt[:, 0::2], rowf[:])
        nc.vector.tensor_copy(wt[:, 1::2], fdv[:])
        WT_d = dr.tile([2 * 640], i16, tag="WT_d")
        nc.sync.dma_start(
            out=WT_d[:].rearrange("(r s t) -> s r t", s=SPC, t=2),
            in_=wt[:].rearrange("s (r t) -> s r t", t=2))
        idxw = sb.tile([P, 2 * 40], i16, tag="idxw")   # col = m*2 + t
        wtd_r = WT_d[:].rearrange("(m q t) -> q m t", q=16, t=2)
        qengs = [nc.sync, nc.scalar]
        for kblk in range(8):
            qengs[kblk % 2].dma_start(
                out=idxw[kblk * 16:(kblk + 1) * 16, :].rearrange(
                    "q (m t) -> q m t", t=2),
                in_=wtd_r)
        idxw3 = sb.tile([P, 120], i16, tag="idxw3")
        nc.vector.tensor_copy(idxw3[:, 0:40], idxw[:, 0::2])
        nc.vector.tensor_scalar(idxw3[:, 40:80], idxw[:, 0::2], 216.0, None, Alu.add)
        nc.vector.tensor_scalar(idxw3[:, 80:120], idxw[:, 0::2], 432.0, None, Alu.add)
        fdivw = sb.tile([P, 40], i16, tag="fdivw")
        nc.vector.tensor_copy(fdivw[:], idxw[:, 1::2])

        # f%64 winner-major [128, 5] straight from fmf via SBUF->SBUF DMAs:
        # winner (pi=(r%4)*32+s, slot=r//4) <- fmf[s, 4*slot + r%4]
        offw = sb.tile([P, 5], f32, tag="offw")
        for r4 in range(4):
            qengs[(r4 + 1) % 2].dma_start(out=offw[r4 * 32:(r4 + 1) * 32, :],
                                          in_=fmf[:, r4::4])

        # ---- phase I: 7 dma_gathers of 64-f32 rows ---------------------
        gath = sb.tile([P, 6 * 320], f32, tag="gath")
        for a, src_ap in enumerate((off_t, shp_t)):
            for c in range(3):
                nc.gpsimd.dma_gather(
                    out_ap=gath[:, (a * 3 + c) * 320:(a * 3 + c + 1) * 320].rearrange(
                        "p (q e) -> p q e", e=64),
                    in_ap=src_ap[:].rearrange("(r e) -> r e", e=64),
                    idxs_ap=idxw3[:, c * 40:(c + 1) * 40],
                    num_idxs=640,
                    num_idxs_reg=640,
                    elem_size=64,
                )
        gatha = sb.tile([P, 5 * 192], f32, tag="gatha")
        nc.gpsimd.dma_gather(
            out_ap=gatha[:].rearrange("p (q e) -> p q e", e=192),
            in_ap=anch_t[:].rearrange("r e -> (r e)").rearrange("(r e) -> r e", e=192),
            idxs_ap=fdivw[:],
            num_idxs=640,
            num_idxs_reg=640,
            elem_size=192,
        )

        # one-hot extraction on DVE: value at column f%64 of each row
        io64 = sb.tile([P, 320], f32, tag="io64")
        nc.gpsimd.iota(io64[:], pattern=[[0, 5], [1, 64]], base=0,
                       channel_multiplier=0, allow_small_or_imprecise_dtypes=True)
        oneh = sb.tile([P, 320], f32, tag="oneh")
        nc.vector.tensor_tensor(
            oneh[:].rearrange("p (q e) -> p q e", e=64),
            io64[:].rearrange("p (q e) -> p q e", e=64),
            offw[:].unsqueeze(2).to_broadcast([P, 5, 64]), Alu.is_equal)
        Wv = sb.tile([P, 45], f32, tag="Wv")
        prod = sb.tile([P, 6 * 320], f32, tag="prod")
        proda = sb.tile([P, 5 * 192], f32, tag="proda")
        oneh4 = oneh[:].rearrange("p (q e) -> p q e", e=64).unsqueeze(1).to_broadcast([P, 6, 5, 64])
        nc.vector.tensor_tensor(
            prod[:].rearrange("p (a q e) -> p a q e", a=6, e=64),
            gath[:].rearrange("p (a q e) -> p a q e", a=6, e=64),
            oneh4, Alu.mult)
        # prod a-dim order is (tensor, channel): a = t*3 + c; slot q inner
        nc.vector.tensor_reduce(
            Wv[:].rearrange("p (q a) -> p a q", a=9)[:, 0:6, :],
            prod[:].rearrange("p (a q e) -> p a q e", a=6, e=64),
            axis=mybir.AxisListType.X, op=Alu.add)
        oneha = oneh[:].rearrange("p (q e) -> p q e", e=64).unsqueeze(2).to_broadcast([P, 5, 3, 64])
        nc.vector.tensor_tensor(
            proda[:].rearrange("p (q c e) -> p q c e", c=3, e=64),
            gatha[:].rearrange("p (q c e) -> p q c e", c=3, e=64),
            oneha, Alu.mult)
        nc.vector.tensor_reduce(
            Wv[:].rearrange("p (q a) -> p q a", a=9)[:, :, 6:9],
            proda[:].rearrange("p (q c e) -> p q c e", c=3, e=64),
            axis=mybir.AxisListType.X, op=Alu.add)

        # winner-major -> sample-major directly via SBUF->SBUF DMAs:
        # winner (pi=(r%4)*32+s, slot=r//4) -> B9[s, r*9+a]
        B9 = sb.tile([SPC, K * 9], f32, tag="B9")
        B9_v = B9[:].rearrange("s (r a) -> s r a", a=9)
        for r4 in range(4):
            eng = nc.scalar if r4 % 2 else nc.sync
            eng.dma_start(out=B9_v[:, r4::4, :],
                          in_=Wv[r4 * 32:(r4 + 1) * 32, :])
        offg = [B9[:, d::9] for d in range(3)]
        shg = [B9[:, 3 + d::9] for d in range(3)]
        anchd = [B9[:, 6 + d::9] for d in range(3)]

        # ---- phase J: det rows [1, score, cz, cy, cx, sz, sy, sx] ------
        HL = sb.tile([SPC, 7 * K], f32, tag="HL")     # hz hy hx lz ly lx vol
        tctr = sb.tile([SPC, K], f32, tag="tctr")
        for d in range(3):
            nc.vector.tensor_tensor(tctr[:], anchd[d], offg[d], Alu.add)
            nc.vector.tensor_scalar(det[:, 2 + d::8], tctr[:], 4.0, None, Alu.mult)
            nc.vector.tensor_tensor(HL[:, d * K:(d + 1) * K], det[:, 2 + d::8], shg[d], Alu.add)
            nc.vector.tensor_tensor(HL[:, (3 + d) * K:(4 + d) * K], det[:, 2 + d::8], shg[d], Alu.subtract)
            nc.vector.tensor_scalar(det[:, 5 + d::8], shg[d], 2.0, None, Alu.mult)
        vtmp = sb.tile([SPC, K], f32, tag="vtmp")
        nc.vector.tensor_tensor(vtmp[:], det[:, 5::8], det[:, 6::8], Alu.mult)
        nc.vector.tensor_tensor(HL[:, 6 * K:7 * K], vtmp[:], det[:, 7::8], Alu.mult)

        # ---- phase K: pairwise IoU on [32, 400] ------------------------
        def brA(col):
            return HL[:, col * K:(col + 1) * K].unsqueeze(2).to_broadcast([SPC, K, K])

        def brB(col):
            return HL[:, col * K:(col + 1) * K].unsqueeze(1).to_broadcast([SPC, K, K])

        dz = sb.tile([SPC, K * K], f32, tag="dz")
        dy = sb.tile([SPC, K * K], f32, tag="dy")
        dx = sb.tile([SPC, K * K], f32, tag="dx")
        tt = sb.tile([SPC, K * K], f32, tag="tt")
        tt2 = sb.tile([SPC, K * K], f32, tag="tt2")
        tt3 = sb.tile([SPC, K * K], f32, tag="tt3")
        tts = [tt, tt2, tt3]
        for d, dd in enumerate((dz, dy, dx)):
            dv = dd[:].rearrange("s (i j) -> s i j", j=K)
            tv = tts[d][:].rearrange("s (i j) -> s i j", j=K)
            nc.vector.tensor_tensor(dv, brA(d), brB(d), Alu.min)
            nc.vector.tensor_tensor(tv, brA(3 + d), brB(3 + d), Alu.max)
            nc.gpsimd.tensor_tensor(dd[:], dd[:], tts[d][:], Alu.subtract)
            nc.gpsimd.tensor_scalar(dd[:], dd[:], 0.0, None, Alu.max)
        inter = dz
        nc.vector.tensor_tensor(inter[:], dz[:], dy[:], Alu.mult)
        nc.vector.tensor_tensor(inter[:], inter[:], dx[:], Alu.mult)
        uni = dy
        uv = uni[:].rearrange("s (i j) -> s i j", j=K)
        nc.vector.tensor_tensor(uv, brA(6), brB(6), Alu.add)
        nc.vector.tensor_tensor(uni[:], uni[:], inter[:], Alu.subtract)
        nc.vector.tensor_scalar(uni[:], uni[:], 1e-8, None, Alu.max)
        rec = dx
        nc.vector.reciprocal(rec[:], uni[:])
        iou = tts[1]
        nc.vector.tensor_tensor(iou[:], inter[:], rec[:], Alu.mult)

        negM = sb.tile([SPC, K * K], f32, tag="negM")
        nc.vector.tensor_scalar(negM[:], iou[:], NMS_THRESH, -1.0, Alu.is_gt, Alu.mult)
        nc.gpsimd.memset(negM[:, 0::K + 1], 0.0)

        # ---- phase L: greedy NMS, 20 sequential steps ------------------
        supp = sb.tile([SPC, K], f32, tag="supp")
        negk = sb.tile([SPC, K], f32, tag="negk")
        nc.gpsimd.memset(supp[:], 0.0)
        for i in range(K):
            nc.vector.scalar_tensor_tensor(
                negk[:, i:i + 1], supp[:, i:i + 1], 1.0, cand[:, i:i + 1],
                Alu.subtract, Alu.mult,
            )
            nc.vector.scalar_tensor_tensor(
                supp[:], negM[:, i * K:(i + 1) * K], negk[:, i:i + 1], supp[:],
                Alu.mult, Alu.max,
            )
        kept = negk
        nc.vector.tensor_scalar(kept[:], negk[:], -1.0, None, Alu.mult)

        # ---- phase M: place rows by rank via local_scatter -------------
        incl = sb.tile([SPC, K], f32, tag="incl")
        nc.vector.tensor_tensor_scan(incl[:], kept[:], kept[:], 0.0, Alu.add, Alu.bypass)
        grow = sb.tile([SPC, K], f32, tag="grow")
        nc.vector.tensor_tensor(grow[:], kept[:], incl[:], Alu.mult)
        nc.vector.tensor_scalar(grow[:], grow[:], 1.0, None, Alu.subtract)
        growbc = sb.tile([SPC, K * 16], f32, tag="growbc")
        nc.scalar.copy(growbc[:].rearrange("s (i x) -> s i x", x=16),
                       grow[:].unsqueeze(2).to_broadcast([SPC, K, 16]))
        xio = sb.tile([SPC, K * 16], f32, tag="xio")
        nc.gpsimd.iota(xio[:], pattern=[[0, K], [1, 16]], base=0,
                       channel_multiplier=0, allow_small_or_imprecise_dtypes=True)
        idxo = sb.tile([SPC, K * 16], i16, tag="idxo")
        nc.vector.scalar_tensor_tensor(idxo[:], growbc[:], 16.0, xio[:],
                                       Alu.mult, Alu.add)
        out160 = sb.tile([SPC, 160], f32, tag="out160")
        nc.gpsimd.local_scatter(out160[:].bitcast(u16), det[:].bitcast(u16),
                                idxo[:], channels=SPC, num_elems=320,
                                num_idxs=320)
        io20 = sb.tile([SPC, K], f32, tag="io20")
        nc.gpsimd.iota(io20[:], pattern=[[1, K]], base=0, channel_multiplier=0,
                       allow_small_or_imprecise_dtypes=True)
        mask20 = sb.tile([SPC, K], f32, tag="mask20")
        nc.vector.tensor_scalar(mask20[:], io20[:], incl[:, K - 1:K], None, Alu.is_lt)
        mask160 = sb.tile([SPC, 160], f32, tag="mask160")
        nc.scalar.copy(mask160[:].rearrange("s (r c) -> s r c", c=8),
                       mask20[:].unsqueeze(2).to_broadcast([SPC, K, 8]))
        outf = sb.tile([SPC, 160], f32, tag="outf")
        nc.vector.tensor_tensor(outf[:], out160[:], mask160[:], Alu.mult)
        nc.vector.scalar_tensor_tensor(outf[:], mask160[:], 1.0,
                                       outf[:], Alu.subtract, Alu.add)
        nc.sync.dma_start(
            out=out_t[:, 0:10, :].rearrange("s r c -> s (r c)"), in_=outf[:, 0:80])
        nc.scalar.dma_start(
            out=out_t[:, 10:K, :].rearrange("s r c -> s (r c)"), in_=outf[:, 80:160])

    nc.compile()
    return nc


def _get_nc():
    if "nc" not in _CACHE:
        _CACHE["nc"] = _build_program()
    return _CACHE["nc"]


def make_in_maps(cls, shape, offset):
    cls = np.ascontiguousarray(np.asarray(cls, dtype=np.float32)).reshape(256, A)
    shape = np.ascontiguousarray(np.asarray(shape, dtype=np.float32)).reshape(256, 3 * A)
    offset = np.ascontiguousarray(np.asarray(offset, dtype=np.float32)).reshape(256, 3 * A)
    in_maps = []
    for c in range(NCORES):
        sl = slice(c * SPC, (c + 1) * SPC)
        in_maps.append({
            "cls": np.ascontiguousarray(cls[sl]),
            "shp": np.ascontiguousarray(shape[sl].reshape(-1)),
            "off": np.ascontiguousarray(offset[sl].reshape(-1)),
        })
    return in_maps


def kernel(cls, shape, offset, _trace=False):
    from concourse.bass_utils import run_bass_kernel_spmd

    nc = _get_nc()
    in_maps = make_in_maps(cls, shape, offset)
    try:
        res = run_bass_kernel_spmd(
            nc, in_maps, core_ids=list(range(NCORES)), trace=_trace)
    except (ImportError, ModuleNotFoundError):
        # NTFF profiling hook unavailable in this environment
        res = run_bass_kernel_spmd(
            nc, in_maps, core_ids=list(range(NCORES)), trace=False)
    out = np.concatenate([res.results[c]["out"] for c in range(NCORES)], axis=0)
    _CACHE["exec_time_ns"] = res.exec_time_ns
    return out.astype(np.float32)
